# revision 1
# baseline (speedup 1.0000x reference)
"""BinNorm (sum-of-sigmoids row normalization via root-find) for Trainium2.

Math: for each row x of shape [256], find nu s.t. sum(sigmoid(x + nu)) == 64,
then output sigmoid(x + nu).  The reference finds nu by a branch-lattice
bisection whose final bracket width is ~6.8e-5 (it quantizes nu to the bracket
midpoint).  Any nu within that quantization radius of the true root produces
outputs within ~1e-5 absmax of the reference, below the fp32 reordering noise
floor of the reference itself (~1.7e-5).

Kernel algorithm per row:
  1. mean/var via bn_stats -> quadratic-poly initializer nu0 (max err ~0.03)
  2. Newton step   (sigmoid ACT pass with row-accumulate f; DVE sum sigma^2)
  3. chord step    (one more sigmoid pass, reuse the Newton reciprocal slope)
  4. output pass   sigmoid(x + nu2), batched per store block: x+nu2 pre-added
     on the idle GPSIMD engine, one wide sigmoid on ACT
Eval sigmoids are single ACT instructions over [128, 256] tiles using the
per-partition bias + accum_out features.

Sharding: pure data parallel over rows, 8 cores x 2048 rows.
"""

import os as _os
import numpy as np

_CORES = 8
_B, _D = 16384, 256
_BC = _B // _CORES          # rows per core
_P = 128                    # partitions
_T = _BC // _P              # 16 row-tiles per core

# per-group tile counts (first groups small to shorten the startup chain)
_GROUPS = tuple(int(v) for v in _os.environ.get(
    "BK_GROUPS", "1,1,1,1,2,2,2,2,1,1,1,1").split(","))
_SCR_BUFS = int(_os.environ.get("BK_SCR_BUFS", "16"))
# input/output DMA block sizes (in 128-row tiles); loads front-loaded small,
# stores tail-loaded small.  width>=2 out blocks get a batched output pass.
_IN_BLOCKS = tuple(int(v) for v in _os.environ.get(
    "BK_IN_BLOCKS", "1,1,2,2,2,4,2,2").split(","))
_OUT_BLOCKS = tuple(int(v) for v in _os.environ.get(
    "BK_OUT_BLOCKS", "4,2,2,2,2,2,1,1").split(","))
_PRE_ENG = _os.environ.get("BK_PRE_ENG", "gpsimd")  # engine for x+nu pre-adds
_SCHEME = _os.environ.get("BK_SCHEME", "newton2")     # halley | newton2
_CU_ENG = _os.environ.get("BK_CU_ENG", "vector")     # engine for sigma^3
_BN_GROUP = _os.environ.get("BK_BN_GROUP", "0") == "1"
_LOOKAHEAD = int(_os.environ.get("BK_LOOKAHEAD", "2"))
_POLY_GP = _os.environ.get("BK_POLY_GP", "0") == "1"
_SW_LOADS = int(_os.environ.get("BK_SW_LOADS", "0"))
_ACT_STORES = int(_os.environ.get("BK_ACT_STORES", "0"))
_HALLEY_SET = set(int(v) for v in _os.environ.get("BK_HALLEY_SET", "1,3,5,7,8,9,10,11").split(",") if v)

# nu0 = C0 + C1*m + C2*v + C3*m^2 + C4*m*v + C5*v^2  (m=row mean, v=row var),
# least-squares fit of the true root over N(0,1) rows.
_C = (-1.097386107696299, -1.0174597913968035, -0.24531199751746788,
      0.010321566224828467, 0.005161273657493432, 0.027572120704527067)

_KF = 64.0                  # target sum

_cache: dict = {}


def _build_nc():
    from contextlib import ExitStack
    import concourse.bacc as bacc
    import concourse.mybir as mybir
    import concourse.tile as tile

    f32 = mybir.dt.float32
    SIG = mybir.ActivationFunctionType.Sigmoid
    A = mybir.AluOpType

    assert sum(_IN_BLOCKS) == _T and sum(_OUT_BLOCKS) == _T
    assert sum(_GROUPS) == _T

    nc = bacc.Bacc(
        "TRN2",
        target_bir_lowering=False,
        debug=False,
        enable_asserts=False,
        num_devices=_CORES,
    )
    x = nc.dram_tensor("x", [_BC, _D], f32, kind="ExternalInput").ap()
    y = nc.dram_tensor("y", [_BC, _D], f32, kind="ExternalOutput").ap()

    with tile.TileContext(nc) as tc, ExitStack() as ctx:
        xp = ctx.enter_context(tc.tile_pool(name="xp", bufs=1))
        sp = ctx.enter_context(tc.tile_pool(name="sp", bufs=_SCR_BUFS))
        op = ctx.enter_context(tc.tile_pool(name="op", bufs=1))
        st = ctx.enter_context(tc.tile_pool(name="st", bufs=1))

        pre_eng = nc.gpsimd if _PRE_ENG == "gpsimd" else nc.vector
        cu_eng = nc.gpsimd if _CU_ENG == "gpsimd" else nc.vector

        # warmup: trigger the sigmoid table load before any data arrives
        wz = st.tile([_P, 1], f32, tag="wz", name="wz")
        nc.vector.memset(wz[:], 0.0)
        wo = st.tile([_P, 1], f32, tag="wo", name="wo")
        nc.scalar.activation(wo[:], wz[:], SIG, bias=wz[:])

        # blocked loads: xt[t] are column views into the block tiles
        xt = [None] * _T
        xwhere = [None] * _T
        t = 0
        for b, w in enumerate(_IN_BLOCKS):
            blk = xp.tile([_P, w * _D], f32, tag=f"xb{b}", name=f"xb{b}")
            src = x[t * _P:(t + w) * _P, :].rearrange("(t p) d -> p t d", p=_P)
            ldeng = nc.gpsimd if b < _SW_LOADS else nc.sync
            ldeng.dma_start(blk[:].rearrange("p (t d) -> p t d", d=_D), src)
            for j in range(w):
                xt[t + j] = blk[:, (j * _D):(j + 1) * _D]
                xwhere[t + j] = (blk, j)
            t += w

        # out block tiles; a block's output pass is emitted once every tile's
        # nu2 is known (nu2col[t] below)
        oblk = []           # [blk, t0, w]
        t = 0
        for b, w in enumerate(_OUT_BLOCKS):
            blk = op.tile([_P, w * _D], f32, tag=f"ob{b}", name=f"ob{b}")
            oblk.append([blk, t, w])
            t += w

        nu2col = [None] * _T      # per-tile [P,1] view of its group's nu2

        def emit_ready_outputs():
            while oblk and all(nu2col[t] is not None
                               for t in range(oblk[0][1],
                                              oblk[0][1] + oblk[0][2])):
                blk, t0, w = oblk.pop(0)
                if w >= 2:
                    pre = sp.tile([_P, w * _D], f32, tag="pre",
                                  name=f"pre_{t0}")
                    for j in range(w):
                        pre_eng.tensor_scalar_add(
                            pre[:, j * _D:(j + 1) * _D], xt[t0 + j],
                            nu2col[t0 + j])
                    nc.scalar.activation(blk[:], pre[:], SIG)
                else:
                    for j in range(w):
                        nc.scalar.activation(
                            blk[:, j * _D:(j + 1) * _D], xt[t0 + j], SIG,
                            bias=nu2col[t0 + j])
                dst = y[t0 * _P:(t0 + w) * _P, :].rearrange(
                    "(t p) d -> p t d", p=_P)
                steng = nc.scalar if (t0 + w > _T - _ACT_STORES) else nc.sync
                steng.dma_start(dst, blk[:].rearrange("p (t d) -> p t d",
                                                      d=_D))

        group_t0 = []
        _acc = 0
        for G in _GROUPS:
            group_t0.append(_acc)
            _acc += G

        nu0_of = {}

        def emit_init(g):
            G = _GROUPS[g]
            t0 = group_t0[g]

            def stile(tag, w=G):
                return st.tile([_P, w], f32, tag=tag, name=tag)

            # ---- moments ----
            agg = st.tile([_P, 2 * G], f32, tag=f"agg{g}", name=f"agg{g}")
            aggv = agg[:].rearrange("p (c g) -> p c g", g=G)  # [P,2,G]
            xb0, xc0 = xwhere[t0]
            xbN, xcN = xwhere[t0 + G - 1]
            if _BN_GROUP and G >= 2 and xb0 is xbN and xcN == xc0 + G - 1:
                bn6 = st.tile([_P, 6 * G], f32, tag=f"bn6_{g}",
                              name=f"bn6_{g}")
                src3 = xb0[:, xc0 * _D:(xc0 + G) * _D].rearrange(
                    "p (t d) -> p t d", d=_D)
                nc.vector.bn_stats(
                    bn6[:].rearrange("p (t c) -> p t c", c=6), src3)
                bn6v = bn6[:].rearrange("p (t c) -> p t c", c=6)
                for j in range(G):
                    nc.vector.bn_aggr(aggv[:, :, j], bn6v[:, j, :])
            else:
                for j in range(G):
                    bn6 = st.tile([_P, 6], f32, tag=f"bn6_{g}_{j}",
                                  name=f"bn6_{g}_{j}")
                    nc.vector.bn_stats(bn6[:], xt[t0 + j])
                    nc.vector.bn_aggr(aggv[:, :, j], bn6[:])
            m1 = aggv[:, 0, :]   # [P,G] mean
            vv = aggv[:, 1, :]   # [P,G] var

            # ---- initializer poly (dep depth 4) ----
            peng = pre_eng if _POLY_GP else nc.vector
            t1 = stile(f"t1_{g}")
            peng.tensor_scalar(t1[:], m1, _C[3], _C[1], A.mult, A.add)
            t4 = stile(f"t4_{g}")
            peng.tensor_scalar(t4[:], vv, _C[5], _C[2], A.mult, A.add)
            t2 = stile(f"t2_{g}")
            nc.vector.scalar_tensor_tensor(t2[:], vv, _C[4], t1[:], A.mult, A.add)
            t5 = stile(f"t5_{g}")
            nc.vector.tensor_mul(t5[:], t4[:], vv)
            t3 = stile(f"t3_{g}")
            nc.vector.tensor_mul(t3[:], t2[:], m1)
            nu0 = stile(f"nu0_{g}")
            nc.vector.scalar_tensor_tensor(nu0[:], t3[:], _C[0], t5[:],
                                           A.add, A.add)

            nu0_of[g] = nu0

        def emit_compute(g):
            G = _GROUPS[g]
            t0 = group_t0[g]
            nu0 = nu0_of[g]

            def stile(tag, w=G):
                return st.tile([_P, w], f32, tag=tag, name=tag)

            if _SCHEME == "halley" or g in _HALLEY_SET:
                # ---- single eval pass: S1=sum s, S2=sum s^2, S3=sum s^3 ----
                S1 = stile(f"S1_{g}")
                S2 = stile(f"S2_{g}")
                S3 = stile(f"S3_{g}")
                for j in range(G):
                    scr = sp.tile([_P, _D], f32, tag="scr", name=f"scr_{g}_{j}")
                    nc.scalar.activation(scr[:], xt[t0 + j], SIG,
                                         bias=nu0[:, j:j + 1],
                                         accum_out=S1[:, j:j + 1])
                    sq = sp.tile([_P, _D], f32, tag="sq", name=f"sq_{g}_{j}")
                    nc.vector.scalar_tensor_tensor(
                        sq[:], scr[:], 0.0, scr[:], A.add, A.mult,
                        accum_out=S2[:, j:j + 1])
                    cu = sp.tile([_P, _D], f32, tag="cu", name=f"cu_{g}_{j}")
                    cu_eng.scalar_tensor_tensor(
                        cu[:], sq[:], 0.0, scr[:], A.add, A.mult,
                        accum_out=S3[:, j:j + 1])
                # ---- Halley: nu2 = nu0 - f*fp / (fp^2 - f*fpp/2) ----
                fp = stile(f"fp_{g}")
                nc.vector.tensor_sub(fp[:], S1[:], S2[:])
                u6 = stile(f"u6_{g}")
                nc.vector.scalar_tensor_tensor(u6[:], S2[:], -3.0, S1[:],
                                               A.mult, A.add)
                fpp = stile(f"fpp_{g}")
                nc.vector.scalar_tensor_tensor(fpp[:], S3[:], 2.0, u6[:],
                                               A.mult, A.add)
                n1 = stile(f"n1_{g}")
                nc.vector.scalar_tensor_tensor(n1[:], S1[:], -_KF, fp[:],
                                               A.add, A.mult)
                d1 = stile(f"d1_{g}")
                nc.vector.tensor_mul(d1[:], fp[:], fp[:])
                d2 = stile(f"d2_{g}")
                nc.vector.scalar_tensor_tensor(d2[:], S1[:], -_KF, fpp[:],
                                               A.add, A.mult)
                den = stile(f"den_{g}")
                nc.vector.scalar_tensor_tensor(den[:], d2[:], -0.5, d1[:],
                                               A.mult, A.add)
                rec = stile(f"rec_{g}")
                nc.vector.reciprocal(rec[:], den[:])
                stp = stile(f"stp_{g}")
                nc.vector.tensor_mul(stp[:], n1[:], rec[:])
                nu2 = stile(f"nu2_{g}")
                nc.vector.tensor_sub(nu2[:], nu0[:], stp[:])
            else:
                # ---- Newton step: nu1 = nu0 - (f0-K)/(f0-q0) ----
                f0 = stile(f"f0_{g}")
                q0 = stile(f"q0_{g}")
                for j in range(G):
                    scr = sp.tile([_P, _D], f32, tag="scr", name=f"scr_{g}_{j}")
                    nc.scalar.activation(scr[:], xt[t0 + j], SIG,
                                         bias=nu0[:, j:j + 1],
                                         accum_out=f0[:, j:j + 1])
                    sq = sp.tile([_P, _D], f32, tag="sq", name=f"sq_{g}_{j}")
                    nc.vector.scalar_tensor_tensor(
                        sq[:], scr[:], 0.0, scr[:], A.add, A.mult,
                        accum_out=q0[:, j:j + 1])
                fp = stile(f"fp_{g}")
                nc.vector.tensor_sub(fp[:], f0[:], q0[:])
                rp = stile(f"rp_{g}")
                nc.vector.reciprocal(rp[:], fp[:])
                stp = stile(f"stp_{g}")
                nc.vector.scalar_tensor_tensor(stp[:], f0[:], -_KF, rp[:],
                                               A.add, A.mult)
                nu1 = stile(f"nu1_{g}")
                nc.vector.tensor_sub(nu1[:], nu0[:], stp[:])

                # ---- chord step: nu2 = nu1 - (f1-K)*rp ----
                f1 = stile(f"f1_{g}")
                for j in range(G):
                    scr3 = sp.tile([_P, _D], f32, tag="scr3",
                                   name=f"scr3_{g}_{j}")
                    nc.scalar.activation(scr3[:], xt[t0 + j], SIG,
                                         bias=nu1[:, j:j + 1],
                                         accum_out=f1[:, j:j + 1])
                stp1 = stile(f"stp1_{g}")
                nc.vector.scalar_tensor_tensor(stp1[:], f1[:], -_KF, rp[:],
                                               A.add, A.mult)
                nu2 = stile(f"nu2_{g}")
                nc.vector.tensor_sub(nu2[:], nu1[:], stp1[:])

            for j in range(G):
                nu2col[t0 + j] = nu2[:, j:j + 1]
            emit_ready_outputs()


        for g in range(min(_LOOKAHEAD, len(_GROUPS))):
            emit_init(g)
        for g in range(len(_GROUPS)):
            la = g + _LOOKAHEAD
            if la < len(_GROUPS):
                emit_init(la)
            emit_compute(g)
        assert not oblk

    nc.compile()
    return nc


def _get_nc():
    if "nc" not in _cache:
        _cache["nc"] = _build_nc()
    return _cache["nc"]


def kernel(x: np.ndarray) -> np.ndarray:
    from concourse.bass_utils import run_bass_kernel_spmd

    x = np.ascontiguousarray(x, dtype=np.float32)
    assert x.shape == (_B, _D), x.shape

    nc = _get_nc()
    in_maps = [{"x": x[i * _BC:(i + 1) * _BC]} for i in range(_CORES)]
    res = run_bass_kernel_spmd(nc, in_maps, list(range(_CORES)))
    out = np.concatenate([res.results[i]["y"] for i in range(_CORES)], axis=0)
    return out.astype(np.float32)



# revision 19
# speedup vs baseline: 1.4882x; 1.4882x over previous
"""BinNorm (sum-of-sigmoids row normalization via root-find) for Trainium2.

Math: for each row x of shape [256], find nu s.t. sum(sigmoid(x + nu)) == 64,
then output sigmoid(x + nu).  The reference's lattice bisection quantizes nu
to a bracket of width ~6.8e-5; any nu within ~1e-3 of the true root keeps the
output within ~2.5e-4 of the reference (sigmoid slope <= 1/4).

Kernel algorithm per row:
  1. row moments via bn_stats  ->  nu0 = c0 + c1*m + (c2 + c3*v)*v
     (least-squares fit of the true root over N(0,1) rows; max err ~0.038)
  2. one Newton step with a fitted reciprocal slope:
        f0  = sum sigmoid(x + nu0)
        nu1 = nu0 - (f0 - 64) * (a0 + a1*v)
     (max residual ~8e-4 in nu -> ~2e-4 in the output)
  3. output pass  sigmoid(x + nu1)

Scheduling: the 16 row-tiles per core stream through a software pipeline of
"units" (1-2 tiles).  Per unit: [optional x+nu0 pre-add] -> eval sigmoid ->
f0 row-sum -> delta -> output sigmoid -> store.  Stats+init-polynomials are
emitted in batches a little ahead of their consumers.  Unit types balance
ACT / DVE / Pool / DMA (the DMA roofline is ~11.65us for 2 MiB in + out):
  b: Pool (or DVE-broadcast) pre-add + one wide ACT sigmoid + DVE reduce
  c: per-tile ACT sigmoid with bias=nu0 + DVE reduce  (no pre-add)
  a: per-tile ACT sigmoid with bias + accum_out f0    (no DVE reduce)
Outputs: 'w' = pre-add delta then one wide sigmoid; 'p' = per-tile sigmoid
with bias (delta on pre for type b, nu0+delta on x otherwise).

Sharding: pure data parallel over rows, 8 cores x 2048 rows.
"""

import os as _os
import numpy as np

_CORES = 8
_B, _D = 16384, 256
_BC = _B // _CORES          # rows per core
_P = 128                    # partitions
_T = _BC // _P              # 16 row-tiles per core

# nu0 = C[0] + C[1]*m + (C[2] + C[3]*v)*v   (m=row mean, v=row var)
_C = (-1.1054261909417549, -1.0002364201254597,
      -0.2275464721729869, 0.0177988072676918)
# 1/f'(root) ~= G[0] + G[1]*v
_G = (0.02112157406163301, 0.0033098367152893152)
_KF = 64.0

# ---- schedule knobs ------------------------------------------------------
# units: <eval><width><fix>; eval a (ACT accum f0) | c (per-tile sigmoid +
# DVE row-sum); fix d (DVE in-place affine) | a (ACT sigmoid bias=nu2) |
# w (pre-add nu2 + wide ACT sigmoid)
_UNITS = _os.environ.get(
    "BK_UNITS", "a1d,a1d,a2d,a2d,a2d,a2d,a2d,a2d,a2d")
_IN_BLOCKS = tuple(int(v) for v in _os.environ.get(
    "BK_IN_BLOCKS", "1,1,2,2,2,2,2,4").split(","))
_OUT_BLOCKS = tuple(int(v) for v in _os.environ.get(
    "BK_OUT_BLOCKS", "2,2,2,2,2,2,2,2").split(","))
# poly batches: <width-in-tiles><engine d|p>
_POLY = _os.environ.get("BK_POLY", "1d,1d,2p,2p,2p,2p,2p,2p,2p")
_DELTA_ENG = _os.environ.get("BK_DELTA_ENG", "ppppppppp")   # per unit
_EVP = _os.environ.get("BK_EVP", "ppppppppp")               # per unit (b)
_OUP = _os.environ.get("BK_OUP", "ppppppppp")               # per unit (w)
_SLA = int(_os.environ.get("BK_SLA", "5"))    # stats lookahead (tiles)
_LAG = int(_os.environ.get("BK_LAG", "1"))    # stage2 lag (units)
_NEWTON = _os.environ.get("BK_NEWTON", "1") == "1"

_cache: dict = {}


def _build_nc():
    from contextlib import ExitStack
    import concourse.bacc as bacc
    import concourse.bass as bass
    import concourse.mybir as mybir
    import concourse.tile as tile

    f32 = mybir.dt.float32
    SIG = mybir.ActivationFunctionType.Sigmoid
    A = mybir.AluOpType
    AX = mybir.AxisListType

    units = []
    for tok in _UNITS.split(","):
        units.append((tok[0], int(tok[1]), tok[2]))  # (type, width, out)
    NU = len(units)
    assert sum(w for _, w, _ in units) == _T
    unit_t0 = []
    _acc = 0
    for (_ty, w, _om) in units:
        unit_t0.append(_acc)
        _acc += w

    pbatches = []
    _acc = 0
    for tok in _POLY.split(","):
        pbatches.append((_acc, int(tok[:-1]), tok[-1]))
        _acc += int(tok[:-1])
    assert _acc == _T
    assert sum(_IN_BLOCKS) == _T and sum(_OUT_BLOCKS) == _T

    nc = bacc.Bacc(
        "TRN2",
        target_bir_lowering=False,
        debug=False,
        enable_asserts=False,
        num_devices=_CORES,
    )
    x = nc.dram_tensor("x", [_BC, _D], f32, kind="ExternalInput").ap()
    y = nc.dram_tensor("y", [_BC, _D], f32, kind="ExternalOutput").ap()

    def bcast(ap2d, g, d=_D):
        """[P, g] AP -> [P, g, d] stride-0 broadcast view."""
        return bass.AP(ap2d.tensor, ap2d.offset,
                       [ap2d.ap[0], [ap2d.ap[1][0], g], [0, d]])

    def widen(col, g):
        """[P, 1] column AP -> contiguous [P, g] AP."""
        return bass.AP(col.tensor, col.offset,
                       [col.ap[0], [col.ap[1][0], g]])

    with tile.TileContext(nc) as tc, ExitStack() as ctx:
        xp = ctx.enter_context(tc.tile_pool(name="xp", bufs=1))
        pp = ctx.enter_context(tc.tile_pool(name="pp", bufs=1))
        op = ctx.enter_context(tc.tile_pool(name="op", bufs=1))
        st = ctx.enter_context(tc.tile_pool(name="st", bufs=1))

        eng = {"d": nc.vector, "p": nc.gpsimd}

        # warmup: trigger the sigmoid table load before any data arrives
        wz = st.tile([_P, 1], f32, tag="wz", name="wz")
        nc.vector.memset(wz[:], 0.0)
        wo = st.tile([_P, 1], f32, tag="wo", name="wo")
        nc.scalar.activation(wo[:], wz[:], SIG, bias=wz[:])

        # ---- blocked loads ----
        xt = [None] * _T            # [P, D] column views per tile
        xcol = [None] * _T          # (block tile, col) per tile
        t = 0
        for b, w in enumerate(_IN_BLOCKS):
            blk = xp.tile([_P, w * _D], f32, tag=f"xb{b}", name=f"xb{b}")
            src = x[t * _P:(t + w) * _P, :].rearrange("(t p) d -> p t d", p=_P)
            nc.sync.dma_start(blk[:].rearrange("p (t d) -> p t d", d=_D), src)
            for j in range(w):
                xt[t + j] = blk[:, (j * _D):(j + 1) * _D]
                xcol[t + j] = (blk, j)
            t += w

        def xwide(t0, w):
            """contiguous [P, w, D] view over x tiles t0..t0+w-1"""
            blk, c0 = xcol[t0]
            blkN, cN = xcol[t0 + w - 1]
            assert blk is blkN and cN == c0 + w - 1, (t0, w)
            return blk[:, c0 * _D:(c0 + w) * _D].rearrange(
                "p (g d) -> p g d", d=_D)

        # ---- store blocks ----
        oblk = []                   # [blk, t0, w]
        ocol = [None] * _T          # (store blk, col) per tile
        t = 0
        for b, w in enumerate(_OUT_BLOCKS):
            blk = op.tile([_P, w * _D], f32, tag=f"ob{b}", name=f"ob{b}")
            oblk.append([blk, t, w])
            for j in range(w):
                ocol[t + j] = (blk, j)
            t += w

        outdone = [False] * _T

        def emit_ready_stores():
            while oblk and all(outdone[t] for t in
                               range(oblk[0][1], oblk[0][1] + oblk[0][2])):
                blk, t0, w = oblk.pop(0)
                dst = y[t0 * _P:(t0 + w) * _P, :].rearrange(
                    "(t p) d -> p t d", p=_P)
                nc.sync.dma_start(dst, blk[:].rearrange("p (t d) -> p t d",
                                                        d=_D))

        # ---- stats + polys per batch ----
        nu0_col = [None] * _T
        gg_col = [None] * _T

        agg_b = [None] * len(pbatches)

        def emit_stats(bi):
            t0, bw, e = pbatches[bi]
            agg = st.tile([_P, 2 * bw], f32, tag=f"agg{bi}", name=f"agg{bi}")
            aggv = agg[:].rearrange("p (c g) -> p c g", g=bw)
            agg_b[bi] = aggv
            for j in range(bw):
                bn6 = st.tile([_P, 6], f32, tag=f"bn6_{bi}_{j}",
                              name=f"bn6_{bi}_{j}")
                nc.vector.bn_stats(bn6[:], xt[t0 + j])
                nc.vector.bn_aggr(aggv[:, :, j], bn6[:])

        def emit_polys(bi):
            t0, bw, e = pbatches[bi]
            pe = eng[e]
            aggv = agg_b[bi]
            m = aggv[:, 0, :]
            v = aggv[:, 1, :]

            def bt(tag):
                return st.tile([_P, bw], f32, tag=tag, name=tag)

            tv = bt(f"tv{bi}")
            pe.tensor_scalar(tv[:], v, _C[3], _C[2], A.mult, A.add)
            tu = bt(f"tu{bi}")
            pe.tensor_tensor(tu[:], tv[:], v, A.mult)
            tw = bt(f"tw{bi}")
            pe.tensor_scalar(tw[:], m, _C[1], _C[0], A.mult, A.add)
            nu0 = bt(f"nu0_{bi}")
            pe.tensor_tensor(nu0[:], tu[:], tw[:], A.add)
            gg = bt(f"gg{bi}")
            pe.tensor_scalar(gg[:], v, _G[1], _G[0], A.mult, A.add)
            for j in range(bw):
                nu0_col[t0 + j] = nu0[:, j:j + 1]
                gg_col[t0 + j] = gg[:, j:j + 1]

        # ---- per-unit compute ----
        # stage1: out0 = sigmoid(x + nu0) written into the store block,
        #         f0 per tile (ACT accum or DVE row-sum), then
        #         dlp = (f0-K)*gg  (= -delta) and c1 = 1 - dlp (= 1+delta).
        # stage2 fix forms:
        #   d: out = (out0*dlp_neg... implemented as (out0*(-dlp)+c1)*out0
        #      via affine_mul_reduce, in place on the store block
        #   a: out = sigmoid(x + nu2) per tile (overwrite), nu2 = nu0 - dlp
        #   w: pre-add nu2 then one wide sigmoid (overwrite)
        dlp_u = [None] * NU
        c1_u = [None] * NU
        nu2_u = [None] * NU

        def stage1(u):
            ty, w, fx = units[u]
            t0 = unit_t0[u]
            if not _NEWTON:
                for j in range(w):
                    ob, oc = ocol[t0 + j]
                    nc.scalar.activation(ob[:, oc * _D:(oc + 1) * _D],
                                         xt[t0 + j], SIG,
                                         bias=nu0_col[t0 + j])
                    outdone[t0 + j] = True
                emit_ready_stores()
                return
            de = eng[_DELTA_ENG[u]]
            f0 = st.tile([_P, w], f32, tag=f"f0_{u}", name=f"f0_{u}")
            if ty == "a":
                for j in range(w):
                    ob, oc = ocol[t0 + j]
                    nc.scalar.activation(
                        ob[:, oc * _D:(oc + 1) * _D], xt[t0 + j], SIG,
                        bias=nu0_col[t0 + j],
                        accum_out=f0[:, j:j + 1])
            else:  # 'c': per-tile sigmoid, f0 via DVE row-sum
                for j in range(w):
                    ob, oc = ocol[t0 + j]
                    nc.scalar.activation(
                        ob[:, oc * _D:(oc + 1) * _D], xt[t0 + j], SIG,
                        bias=nu0_col[t0 + j])
                ob0, oc0 = ocol[t0]
                obN, ocN = ocol[t0 + w - 1]
                assert ob0 is obN and ocN == oc0 + w - 1, (u, t0, w)
                nc.vector.tensor_reduce(
                    f0[:], ob0[:, oc0 * _D:(oc0 + w) * _D].rearrange(
                        "p (g d) -> p g d", d=_D),
                    AX.X, A.add)

            dlp = st.tile([_P, w], f32, tag=f"dl_{u}", name=f"dl_{u}")
            if _DELTA_ENG[u] == "p":
                # Pool has no scalar_tensor_tensor; two-op form
                fk = st.tile([_P, w], f32, tag=f"fk_{u}", name=f"fk_{u}")
                nc.gpsimd.tensor_scalar(fk[:], f0[:], 1.0, -_KF,
                                        A.mult, A.add)
                nc.gpsimd.tensor_tensor(dlp[:], fk[:], widen(gg_col[t0], w),
                                        A.mult)
            else:
                nc.vector.scalar_tensor_tensor(dlp[:], f0[:], -_KF,
                                               widen(gg_col[t0], w),
                                               A.add, A.mult)
            dlp_u[u] = dlp
            if fx in ("d", "p"):
                c1 = st.tile([_P, w], f32, tag=f"c1_{u}", name=f"c1_{u}")
                de.tensor_scalar(c1[:], dlp[:], -1.0, 1.0, A.mult, A.add)
                c1_u[u] = c1
            else:
                nu2 = st.tile([_P, w], f32, tag=f"nu2_{u}", name=f"nu2_{u}")
                de.tensor_tensor(nu2[:], widen(nu0_col[t0], w), dlp[:],
                                 A.subtract)
                nu2_u[u] = nu2

        def stage2(u):
            ty, w, fx = units[u]
            t0 = unit_t0[u]
            if not _NEWTON:
                return
            if fx == "d":
                # delta = -dlp:  out = (out0*dlp + (1-dlp)) * out0, in place
                for j in range(w):
                    ob, oc = ocol[t0 + j]
                    dst = ob[:, oc * _D:(oc + 1) * _D]
                    acc = st.tile([_P, 1], f32, tag=f"fac_{u}_{j}",
                                  name=f"fac_{u}_{j}")
                    nc.vector.affine_mul_reduce(
                        dst, acc[:], dst, dst,
                        dlp_u[u][:, j:j + 1], c1_u[u][:, j:j + 1])
                    outdone[t0 + j] = True
            elif fx == "p":
                # same fix on Pool as two ops via a scratch tile
                for j in range(w):
                    ob, oc = ocol[t0 + j]
                    dst = ob[:, oc * _D:(oc + 1) * _D]
                    tmp = pp.tile([_P, _D], f32, tag=f"ft_{u}_{j}",
                                  name=f"ft_{u}_{j}")
                    nc.gpsimd.tensor_scalar(tmp[:], dst,
                                            dlp_u[u][:, j:j + 1],
                                            c1_u[u][:, j:j + 1],
                                            A.mult, A.add)
                    nc.gpsimd.tensor_tensor(dst, tmp[:], dst, A.mult)
                    outdone[t0 + j] = True
            elif fx == "a":
                for j in range(w):
                    ob, oc = ocol[t0 + j]
                    nc.scalar.activation(ob[:, oc * _D:(oc + 1) * _D],
                                         xt[t0 + j], SIG,
                                         bias=nu2_u[u][:, j:j + 1])
                    outdone[t0 + j] = True
            else:  # 'w'
                pre2 = pp.tile([_P, w * _D], f32, tag=f"pr2_{u}",
                               name=f"pr2_{u}")
                if _OUP[u] == "p":
                    for j in range(w):
                        nc.gpsimd.tensor_scalar_add(
                            pre2[:, j * _D:(j + 1) * _D],
                            xt[t0 + j], nu2_u[u][:, j:j + 1])
                else:
                    nc.vector.tensor_tensor(
                        pre2[:].rearrange("p (g d) -> p g d", d=_D),
                        xwide(t0, w), bcast(nu2_u[u][:], w), A.add)
                ob0, oc0 = ocol[t0]
                obN, ocN = ocol[t0 + w - 1]
                if ob0 is obN and ocN == oc0 + w - 1:
                    nc.scalar.activation(
                        ob0[:, oc0 * _D:(oc0 + w) * _D], pre2[:], SIG)
                else:
                    for j in range(w):
                        ob, oc = ocol[t0 + j]
                        nc.scalar.activation(
                            ob[:, oc * _D:(oc + 1) * _D],
                            pre2[:, j * _D:(j + 1) * _D], SIG)
                for j in range(w):
                    outdone[t0 + j] = True
            emit_ready_stores()

        # ---- pipelined emission ----
        next_sb = 0
        next_pb = 0

        def stats_upto(tile_limit):
            nonlocal next_sb
            while (next_sb < len(pbatches)
                   and pbatches[next_sb][0] < tile_limit):
                emit_stats(next_sb)
                next_sb += 1

        def polys_upto(tile_limit):
            nonlocal next_pb
            while (next_pb < len(pbatches)
                   and pbatches[next_pb][0] < tile_limit):
                stats_upto(pbatches[next_pb][0] + 1)
                emit_polys(next_pb)
                next_pb += 1

        for u in range(NU):
            polys_upto(min(_T, unit_t0[u] + units[u][1]))
            stats_upto(min(_T, unit_t0[u] + units[u][1] + _SLA))
            stage1(u)
            if u >= _LAG:
                stage2(u - _LAG)
        for u in range(max(0, NU - _LAG), NU):
            stage2(u)
        assert not oblk, oblk

    nc.compile()
    return nc


def _get_nc():
    if "nc" not in _cache:
        _cache["nc"] = _build_nc()
    return _cache["nc"]


def kernel(x: np.ndarray) -> np.ndarray:
    from concourse.bass_utils import run_bass_kernel_spmd

    x = np.ascontiguousarray(x, dtype=np.float32)
    assert x.shape == (_B, _D), x.shape

    nc = _get_nc()
    in_maps = [{"x": x[i * _BC:(i + 1) * _BC]} for i in range(_CORES)]
    res = run_bass_kernel_spmd(nc, in_maps, list(range(_CORES)))
    out = np.concatenate([res.results[i]["y"] for i in range(_CORES)], axis=0)
    return out.astype(np.float32)
